# revision 24
# baseline (speedup 1.0000x reference)
"""MultiHeadAttention forward on 8 TRN2 NeuronCores (batch*head parallel).

Problem: S=2048, B=2, E=1024, H=16 heads, D=64. Each core handles one batch
(b = core//4) and 4 consecutive heads ((core%4)*4 ...), as 2 head-pairs.

Final version (286us -> ~230us): fp16 matmul operands everywhere (1 cyc/row
on the PE like bf16 but with a 10-bit mantissa; measured f32r streams ~1.6
cyc/row on HW), fp16 partial outputs (halves the out-DMA), deferred-PV
software pipelining (the PE streams scores(i+1) while the ACT exps strip i,
never stalling on the exp), out_proj deferred one t-quarter (the PE never
waits on the finalize chain), input DMAs split across the SP/Activation HWDGE
queues with wqk first and x in column halves, steady-state out-DMAs issued
from the idle GpSimd SWDGE queue, and the reciprocal broadcast done with a
DRAM-bounce stride-0 DMA. PSUM accumulation stays fp32 throughout.

Measured phase-B steady state: score pair 31ns apart (row groups stream
concurrently), P@V pair chains at ~215ns/512 rows; ~1.04us/iteration vs the
1.11us exp ACT — the kernel is tensor-engine-bound end-to-end at ~80% PE
occupancy, with the remainder being input-DMA ramp (~14us), the closing
reciprocal/out_proj chain (~10us), and the final sync barrier (~7us).

Per-core program:
  Phase A: QKV projection. Host pre-transposes x and weight slices so the
    contraction dim (E) lands on SBUF partitions. Q^T/K^T in feature-major
    [f, s] layout; V in natural [s, d] layout with an appended ones column
    (the softmax denominator drops out of the P@V matmul). Only pair 0's
    Q/K + V precede phase B; pair 1's Q/K are emitted between the pairs.
  Phase B: per head-pair, per t-quarter (512): row-packed K=64 score matmuls
    (heads at array rows 0-63/64-127 run concurrently), one ACT exp over the
    combined [128, 1024] PSUM strip (scale=1/8 folded in) producing fp16 P,
    then P@V accumulation with [V|1] stationary deferred one iteration so
    the PE streams scores(i+1) while the ACT runs exp(i).
  Finalize (per pair/tq, off critical path): PSUM->SBUF staging copy,
    batched DVE reciprocal over the 8 denominator rows (partition-shuffled
    via DMA), broadcast via stride-0 DMA, DVE multiply (casts attn to fp16).
  Phase C: out_proj partials per pair; host sums the 2x4 fp16 partials per
    batch in fp32 and adds out_proj_bias.
"""
import os
import sys

if "/opt/trn_rl_repo" not in sys.path:
    sys.path.insert(0, "/opt/trn_rl_repo")

import numpy as np

import concourse.bass as bass
import concourse.tile as tile
from concourse import mybir
from concourse.bass_utils import run_bass_kernel_spmd

_LDWOPT = os.environ.get("LDWOPT", "0") == "1"
if _LDWOPT:
    import concourse.bass_utils as _bu

    _orig_run_command = _bu.run_command

    def _run_command_ldwopt(argv, **kw):
        argv = ["--enable-ldw-opt=true" if a == "--enable-ldw-opt=false" else a
                for a in argv]
        return _orig_run_command(argv, **kw)

    _bu.run_command = _run_command_ldwopt

# BCAST mode: "dma" = stride-0 SBUF->SBUF DMA; "pe" = ones-matmul on the PE.
_BCAST = os.environ.get("BCAST", "dma")

S = 2048
B = 2
E = 1024
H = 16
D = 64
N_CORES = 8
F32 = mybir.dt.float32
F16 = mybir.dt.float16
EXP = mybir.ActivationFunctionType.Exp
SCALING = float(D) ** -0.5

NSCH = S // 128   # 16 s-chunks
NSB = S // 512    # 4 s-blocks
NEC = E // 128    # 8 e-chunks


def _split_excess_waits(nc, limit=1):
    """This walrus build accepts at most 2 sync-wait commands per instruction;
    hoist excess waits onto preceding same-engine NOPs (queue order preserves
    semantics)."""
    ctr = 0
    for f in nc.m.functions:
        for blk in f.blocks:
            insts = blk.instructions
            if not any(
                i.sync_info and i.sync_info.on_wait and len(i.sync_info.on_wait) > limit
                for i in insts
            ):
                continue
            out = []
            for inst in insts:
                si = inst.sync_info
                if si and si.on_wait and len(si.on_wait) > limit:
                    waits = list(si.on_wait)
                    excess, keep = waits[:-limit], waits[-limit:]
                    for i in range(0, len(excess), limit):
                        ctr += 1
                        nop = mybir.InstNoOp(name=f"waitsplit-nop-{ctr}")
                        nop.engine = inst.engine
                        nop.sync_info = mybir.SyncInfo(
                            on_wait=excess[i : i + limit], on_update=[]
                        )
                        nc.register_instruction(nop, overwrite=True)
                        out.append(nop)
                    si.on_wait = keep
                out.append(inst)
            blk.instructions.clear()
            blk.instructions.extend(out)
    return nc


def _build_nc():
    nc = bass.Bass()
    xT = nc.dram_tensor("xT", [E, S], F16, kind="ExternalInput")
    wqkT = nc.dram_tensor("wqkT", [E, 512], F16, kind="ExternalInput")
    wvT = nc.dram_tensor("wvT", [E, 256], F16, kind="ExternalInput")
    woutT = nc.dram_tensor("woutT", [256, E], F16, kind="ExternalInput")
    bias_qk = nc.dram_tensor("bias_qk", [128, 4], F32, kind="ExternalInput")
    bias_v = nc.dram_tensor("bias_v", [1, 256], F32, kind="ExternalInput")
    outT = nc.dram_tensor("outT", [2, E, S], F16, kind="ExternalOutput")
    recscr = nc.dram_tensor("recscr", [2, 2, 512], F32, kind="Internal")

    with tile.TileContext(nc) as tc:
        with tc.tile_pool(name="wpool", bufs=1) as wpool, \
             tc.tile_pool(name="qkpool", bufs=1) as qkpool, \
             tc.tile_pool(name="vapool", bufs=1) as vapool, \
             tc.tile_pool(name="attnpool", bufs=1) as attnpool, \
             tc.tile_pool(name="ppool", bufs=4) as ppool, \
             tc.tile_pool(name="scpsum", bufs=2, space="PSUM") as scp, \
             tc.tile_pool(name="pvpsum", bufs=1, space="PSUM") as pvp:
            # ---- constants / weights. x + QK weights first (the PE's
            # critical path); spread across issue queues so transfers overlap.
            xt = wpool.tile([128, NEC, S], F16)
            wqk = wpool.tile([128, NEC, 512], F16)
            # wqk first (small, needed by every projection group), then x in
            # column-halves so s-blocks 0-1 can project before x fully lands.
            # wv/bv must land before the V chunks (3rd in PE order).
            for ec in range(NEC):
                eng = nc.sync if ec % 2 == 0 else nc.scalar
                eng.dma_start(
                    out=wqk[:, ec, :], in_=wqkT[bass.ts(ec, 128), :])
            bqk = wpool.tile([128, 4], F32)
            nc.scalar.dma_start(out=bqk, in_=bias_qk[:, :])
            for half in range(2):
                cs = bass.ds(half * 1024, 1024)
                for ec in range(NEC):
                    eng = nc.sync if ec % 2 == 0 else nc.scalar
                    eng.dma_start(out=xt[:, ec, cs],
                                  in_=xT[bass.ts(ec, 128), cs])
                if half == 0:
                    wv = wpool.tile([128, NEC, 256], F16)
                    nc.scalar.dma_start(
                        out=wv, in_=wvT.rearrange("(c p) f -> p c f", p=128))
                    bv = wpool.tile([128, 256], F32)
                    nc.scalar.dma_start(
                        out=bv, in_=bias_v[:, :].to_broadcast([128, 256]))
            wout = wpool.tile([128, 2, E], F16)
            nc.sync.dma_start(
                out=wout, in_=woutT.rearrange("(c p) f -> p c f", p=128))
            ones64 = wpool.tile([128, 64], F16)
            # named scope doubles as a compile-cache buster
            with nc.named_scope(f"init3_ldwopt{int(_LDWOPT)}_bc{_BCAST}"):
                nc.vector.memset(ones64, 1.0)
            onesbc = wpool.tile([1, 64], F16)
            nc.vector.tensor_copy(onesbc, ones64[0:1, :])

            # persistent activations
            qk = qkpool.tile([128, 4, S], F16)        # Q^T (chunks 0-1), K^T (2-3)
            # V natural + ones col, flattened per s-chunk to [4*65 + 68pad]
            # so each head's [V|1] stationary can be read as a 128-col AP
            # (full-width weights enable the PE fast weight load).
            va = vapool.tile([128, NSCH, 328], F16)
            attn = attnpool.tile([128, 2, S], F16)    # attn^T normalized

            def va_hd(i):
                return va[:, i, 0:260].rearrange("p (h c) -> p h c", h=4)

            nc.vector.memset(va[:, :, 260:328], 0.0)
            nc.vector.memset(
                va[:, :, 0:260].rearrange(
                    "p i (h c) -> p i h c", h=4)[:, :, :, 64:65], 1.0)

            with tc.tile_pool(name="apsum", bufs=2, space="PSUM") as apsum, \
                 tc.tile_pool(name="unpool", bufs=4) as unpool, \
                 tc.tile_pool(name="fpool", bufs=3) as fpool, \
                 tc.tile_pool(name="opool", bufs=6) as opool:

                def emit_qk(fc, sbs=range(NSB)):
                    with nc.named_scope(f"proj_qk{fc}"):
                        for sb in sbs:
                            ps = apsum.tile([128, 512], F32, tag="aps")
                            for ec in range(NEC):
                                nc.tensor.matmul(
                                    ps,
                                    wqk[:, ec, bass.ts(fc, 128)],
                                    xt[:, ec, bass.ts(sb, 512)],
                                    start=(ec == 0), stop=(ec == NEC - 1))
                            nc.vector.tensor_scalar(
                                out=qk[:, fc, bass.ts(sb, 512)], in0=ps,
                                scalar1=bqk[:, fc:fc + 1], scalar2=None,
                                op0=mybir.AluOpType.add)

                def emit_v(chunks=range(NSCH)):
                    with nc.named_scope("proj_v"):
                        for i in chunks:
                            ps = apsum.tile([128, 512], F32, tag="aps")
                            for ec in range(NEC):
                                nc.tensor.matmul(
                                    ps[:, 0:256],
                                    xt[:, ec, bass.ts(i, 128)],
                                    wv[:, ec, :],
                                    start=(ec == 0), stop=(ec == NEC - 1))
                            nc.vector.tensor_tensor(
                                out=va_hd(i)[:, :, 0:64],
                                in0=ps[:, 0:256].rearrange(
                                    "p (h d) -> p h d", h=4),
                                in1=bv.rearrange("p (h d) -> p h d", h=4),
                                op=mybir.AluOpType.add)

                def emit_oproj(pair, tq):
                    toff = tq * 512
                    final = (pair == 1 and tq == 3)
                    with nc.named_scope(f"oproj{pair}_{tq}"):
                        for fc in range(NEC):
                            ps = apsum.tile([128, 512], F32, tag="aps")
                            nc.tensor.matmul(
                                ps,
                                wout[:, pair, bass.ts(fc, 128)],
                                attn[:, pair, bass.ds(toff, 512)],
                                start=True, stop=True)
                            ocp = opool.tile([128, 512], F16, tag="ocp")
                            if final and fc % 2 == 1:
                                nc.scalar.copy(ocp, ps)
                            else:
                                nc.vector.tensor_copy(ocp, ps)
                            if final:
                                oeng = nc.sync if fc % 2 == 0 else nc.scalar
                            else:
                                oeng = nc.gpsimd
                            oeng.dma_start(
                                out=outT[pair, bass.ts(fc, 128),
                                         bass.ds(toff, 512)],
                                in_=ocp)

                def emit_pair(pair):
                    hA, hB = 2 * pair, 2 * pair + 1
                    qc = pair       # Q chunk of this pair
                    kc = 2 + pair   # K chunk
                    auxp = apsum
                    if True:
                        for tq in range(4):
                            toff = tq * 512
                            if tq > 0 and not (pair == 1 and tq == 3):
                                # out_proj for the previous t-quarter: its
                                # attn is ready by now, so the PE never
                                # stalls on the finalize chain. For the very
                                # last t-quarter it is instead emitted after
                                # the finalize, giving the PE work to chew
                                # while the closing reciprocal chain runs.
                                emit_oproj(pair, tq - 1)
                            pvA = pvp.tile([128, 512], F32, tag="pvA")
                            pvB = pvp.tile([128, 512], F32, tag="pvB")
                            # software-pipelined: scores/exp for i, P@V for
                            # i-1, so the PE streams scores while ACT runs.
                            ptiles = {}
                            with nc.named_scope(f"scores{pair}_{tq}"):
                                for i in range(NSCH + 1):
                                    if i < NSCH:
                                        sc = scp.tile(
                                            [128, 1024], F32, tag="sc")
                                        nc.tensor.matmul(
                                            sc[:, 0:512],
                                            qk[0:64, kc, bass.ts(i, 128)],
                                            qk[0:64, qc, bass.ds(toff, 512)],
                                            start=True, stop=True)
                                        nc.tensor.matmul(
                                            sc[:, 512:1024],
                                            qk[64:128, kc, bass.ts(i, 128)],
                                            qk[64:128, qc, bass.ds(toff, 512)],
                                            start=True, stop=True)
                                        p = ppool.tile(
                                            [128, 1024], F16, tag="p")
                                        nc.scalar.activation(
                                            p, sc, EXP, scale=SCALING)
                                        ptiles[i] = p
                                    if i > 0:
                                        pp = ptiles.pop(i - 1)
                                        nc.tensor.matmul(
                                            pvA,
                                            va[:, i - 1,
                                               hA * 65:hA * 65 + 128],
                                            pp[:, 0:512],
                                            start=(i == 1), stop=(i == NSCH))
                                        nc.tensor.matmul(
                                            pvB,
                                            va[:, i - 1,
                                               hB * 65:hB * 65 + 128],
                                            pp[:, 512:1024],
                                            start=(i == 1), stop=(i == NSCH))
                            # finalize this t-quarter: stage unnormalized
                            # P@V + sums to SBUF (frees PSUM), reciprocal via
                            # partition shuffle, stride-0 DMA broadcast,
                            # normalize.
                            final = (pair == 1 and tq == 3)
                            with nc.named_scope(f"fin{pair}_{tq}"):
                                un = unpool.tile([65, 2, 512], F32, tag="un")
                                nc.vector.tensor_copy(un[:, 0, :], pvA[0:65, :])
                                nc.vector.tensor_copy(un[:, 1, :], pvB[0:65, :])
                                recin = fpool.tile(
                                    [128, 8], F32, tag="recin")
                                nc.sync.dma_start(
                                    out=recin, in_=un[64:65, :, :])
                                recw = fpool.tile([128, 8], F32, tag="recw")
                                nc.vector.reciprocal(recw, recin)
                                nc.sync.dma_start(
                                    out=recscr[tq % 2], in_=recw)
                                for h in range(2):
                                    prt = h * 64
                                    bcs = opool.tile(
                                        [64, 512], F32, tag="bcs")
                                    nc.sync.dma_start(
                                        out=bcs,
                                        in_=recscr[tq % 2, h:h + 1,
                                                   :].to_broadcast(
                                            [64, 512]))
                                    nc.vector.tensor_mul(
                                        attn[prt:prt + 64, pair,
                                             bass.ds(toff, 512)],
                                        un[0:64, h, :],
                                        bcs)

                emit_qk(2, range(2))
                emit_qk(0, range(2))
                emit_v(range(8))
                emit_qk(2, range(2, 4))
                emit_qk(0, range(2, 4))
                emit_v(range(8, NSCH))
                emit_pair(0)
                emit_qk(1)
                emit_qk(3)
                emit_oproj(0, 3)
                emit_pair(1)
                emit_oproj(1, 2)
                emit_oproj(1, 3)
    _split_excess_waits(nc)
    return nc


_NC_CACHE = None


def _get_nc():
    global _NC_CACHE
    if _NC_CACHE is None:
        _NC_CACHE = _build_nc()
    return _NC_CACHE


def kernel(x, in_proj_weight, in_proj_bias, out_proj_weight, out_proj_bias,
           _run_kwargs=None, _capture=None):
    x = np.asarray(x, dtype=np.float32)
    in_proj_weight = np.asarray(in_proj_weight, dtype=np.float32)
    in_proj_bias = np.asarray(in_proj_bias, dtype=np.float32)
    out_proj_weight = np.asarray(out_proj_weight, dtype=np.float32)
    out_proj_bias = np.asarray(out_proj_bias, dtype=np.float32)

    nc = _get_nc()
    xTb = [np.ascontiguousarray(x[:, b, :].T.astype(np.float16))
           for b in range(B)]

    in_maps = []
    for c in range(N_CORES):
        b = c // 4
        h0 = (c % 4) * 4
        rows = slice(h0 * D, h0 * D + 4 * D)
        wq = in_proj_weight[0:E][rows]          # [256, 1024]
        wk = in_proj_weight[E:2 * E][rows]
        wv_ = in_proj_weight[2 * E:3 * E][rows]
        wqkT = np.ascontiguousarray(
            np.concatenate([wq, wk], axis=0).T.astype(np.float16))
        wvT = np.ascontiguousarray(wv_.T.astype(np.float16))
        woutT = np.ascontiguousarray(
            out_proj_weight[:, rows].T.astype(np.float16))
        bqk = np.concatenate(
            [in_proj_bias[0:E][rows], in_proj_bias[E:2 * E][rows]])
        bias_qk = np.ascontiguousarray(bqk.reshape(4, 128).T)
        bias_v = in_proj_bias[2 * E:3 * E][rows].reshape(1, 256)
        in_maps.append({
            "xT": xTb[b],
            "wqkT": wqkT,
            "wvT": wvT,
            "woutT": woutT,
            "bias_qk": bias_qk,
            "bias_v": np.ascontiguousarray(bias_v),
        })

    res = run_bass_kernel_spmd(nc, in_maps, core_ids=list(range(N_CORES)),
                               **(_run_kwargs or {}))
    if _capture is not None:
        _capture["res"] = res

    out = np.zeros((S, B, E), dtype=np.float32)
    for c in range(N_CORES):
        b = c // 4
        o = res.results[c]["outT"]
        out[:, b, :] += o[0].T.astype(np.float32)
        out[:, b, :] += o[1].T.astype(np.float32)
    out += out_proj_bias
    return out


# revision 25
# speedup vs baseline: 1.0051x; 1.0051x over previous
"""MultiHeadAttention forward on 8 TRN2 NeuronCores (batch*head parallel).

Problem: S=2048, B=2, E=1024, H=16 heads, D=64. Each core handles one batch
(b = core//4) and 4 consecutive heads ((core%4)*4 ...), as 2 head-pairs.

Final version (286us -> ~230us): fp16 matmul operands everywhere (1 cyc/row
on the PE like bf16 but with a 10-bit mantissa; measured f32r streams ~1.6
cyc/row on HW), fp16 partial outputs (halves the out-DMA), deferred-PV
software pipelining (the PE streams scores(i+1) while the ACT exps strip i,
never stalling on the exp), out_proj deferred one t-quarter (the PE never
waits on the finalize chain), input DMAs split across the SP/Activation HWDGE
queues with wqk first and x in column halves, steady-state out-DMAs issued
from the idle GpSimd SWDGE queue, and the reciprocal broadcast done with a
DRAM-bounce stride-0 DMA. PSUM accumulation stays fp32 throughout.

Measured phase-B steady state: score pair 31ns apart (row groups stream
concurrently), P@V pair chains at ~215ns/512 rows; ~1.04us/iteration vs the
1.11us exp ACT — the kernel is tensor-engine-bound end-to-end at ~80% PE
occupancy, with the remainder being input-DMA ramp (~14us), the closing
reciprocal/out_proj chain (~10us), and the final sync barrier (~7us).

Per-core program:
  Phase A: QKV projection. Host pre-transposes x and weight slices so the
    contraction dim (E) lands on SBUF partitions. Q^T/K^T in feature-major
    [f, s] layout; V in natural [s, d] layout with an appended ones column
    (the softmax denominator drops out of the P@V matmul). Only pair 0's
    Q/K + V precede phase B; pair 1's Q/K are emitted between the pairs.
  Phase B: per head-pair, per t-quarter (512): row-packed K=64 score matmuls
    (heads at array rows 0-63/64-127 run concurrently), one ACT exp over the
    combined [128, 1024] PSUM strip (scale=1/8 folded in) producing fp16 P,
    then P@V accumulation with [V|1] stationary deferred one iteration so
    the PE streams scores(i+1) while the ACT runs exp(i).
  Finalize (per pair/tq, off critical path): PSUM->SBUF staging copy,
    batched DVE reciprocal over the 8 denominator rows (partition-shuffled
    via DMA), broadcast via stride-0 DMA, DVE multiply (casts attn to fp16).
  Phase C: out_proj partials per pair; host sums the 2x4 fp16 partials per
    batch in fp32 and adds out_proj_bias.
"""
import os
import sys

if "/opt/trn_rl_repo" not in sys.path:
    sys.path.insert(0, "/opt/trn_rl_repo")

import numpy as np

import concourse.bass as bass
import concourse.tile as tile
from concourse import mybir
from concourse.bass_utils import run_bass_kernel_spmd

_LDWOPT = os.environ.get("LDWOPT", "0") == "1"
if _LDWOPT:
    import concourse.bass_utils as _bu

    _orig_run_command = _bu.run_command

    def _run_command_ldwopt(argv, **kw):
        argv = ["--enable-ldw-opt=true" if a == "--enable-ldw-opt=false" else a
                for a in argv]
        return _orig_run_command(argv, **kw)

    _bu.run_command = _run_command_ldwopt

# BCAST mode: "dma" = stride-0 SBUF->SBUF DMA; "pe" = ones-matmul on the PE.
_BCAST = os.environ.get("BCAST", "dma")

S = 2048
B = 2
E = 1024
H = 16
D = 64
N_CORES = 8
F32 = mybir.dt.float32
F16 = mybir.dt.float16
EXP = mybir.ActivationFunctionType.Exp
SCALING = float(D) ** -0.5

NSCH = S // 128   # 16 s-chunks
NSB = S // 512    # 4 s-blocks
NEC = E // 128    # 8 e-chunks


def _split_excess_waits(nc, limit=1):
    """This walrus build accepts at most 2 sync-wait commands per instruction;
    hoist excess waits onto preceding same-engine NOPs (queue order preserves
    semantics)."""
    ctr = 0
    for f in nc.m.functions:
        for blk in f.blocks:
            insts = blk.instructions
            if not any(
                i.sync_info and i.sync_info.on_wait and len(i.sync_info.on_wait) > limit
                for i in insts
            ):
                continue
            out = []
            for inst in insts:
                si = inst.sync_info
                if si and si.on_wait and len(si.on_wait) > limit:
                    waits = list(si.on_wait)
                    excess, keep = waits[:-limit], waits[-limit:]
                    for i in range(0, len(excess), limit):
                        ctr += 1
                        nop = mybir.InstNoOp(name=f"waitsplit-nop-{ctr}")
                        nop.engine = inst.engine
                        nop.sync_info = mybir.SyncInfo(
                            on_wait=excess[i : i + limit], on_update=[]
                        )
                        nc.register_instruction(nop, overwrite=True)
                        out.append(nop)
                    si.on_wait = keep
                out.append(inst)
            blk.instructions.clear()
            blk.instructions.extend(out)
    return nc


def _build_nc():
    nc = bass.Bass()
    xT = nc.dram_tensor("xT", [E, S], F16, kind="ExternalInput")
    wqkT = nc.dram_tensor("wqkT", [E, 512], F16, kind="ExternalInput")
    wvT = nc.dram_tensor("wvT", [E, 256], F16, kind="ExternalInput")
    woutT = nc.dram_tensor("woutT", [256, E], F16, kind="ExternalInput")
    bias_qk = nc.dram_tensor("bias_qk", [128, 4], F32, kind="ExternalInput")
    bias_v = nc.dram_tensor("bias_v", [1, 256], F32, kind="ExternalInput")
    outT = nc.dram_tensor("outT", [2, E, S], F16, kind="ExternalOutput")
    recscr = nc.dram_tensor("recscr", [2, 2, 512], F32, kind="Internal")

    with tile.TileContext(nc) as tc:
        with tc.tile_pool(name="wpool", bufs=1) as wpool, \
             tc.tile_pool(name="qkpool", bufs=1) as qkpool, \
             tc.tile_pool(name="vapool", bufs=1) as vapool, \
             tc.tile_pool(name="attnpool", bufs=1) as attnpool, \
             tc.tile_pool(name="ppool", bufs=4) as ppool, \
             tc.tile_pool(name="scpsum", bufs=2, space="PSUM") as scp, \
             tc.tile_pool(name="pvpsum", bufs=1, space="PSUM") as pvp:
            # ---- constants / weights. x + QK weights first (the PE's
            # critical path); spread across issue queues so transfers overlap.
            xt = wpool.tile([128, NEC, S], F16)
            wqk = wpool.tile([128, NEC, 512], F16)
            # wqk first (small, needed by every projection group), then x in
            # column-halves so s-blocks 0-1 can project before x fully lands.
            # wv/bv must land before the V chunks (3rd in PE order).
            for ec in range(NEC):
                eng = nc.sync if ec % 2 == 0 else nc.scalar
                eng.dma_start(
                    out=wqk[:, ec, :], in_=wqkT[bass.ts(ec, 128), :])
            bqk = wpool.tile([128, 4], F32)
            nc.scalar.dma_start(out=bqk, in_=bias_qk[:, :])
            for half in range(2):
                cs = bass.ds(half * 1024, 1024)
                for ec in range(NEC):
                    eng = nc.sync if ec % 2 == 0 else nc.scalar
                    eng.dma_start(out=xt[:, ec, cs],
                                  in_=xT[bass.ts(ec, 128), cs])
                if half == 0:
                    wv = wpool.tile([128, NEC, 256], F16)
                    nc.scalar.dma_start(
                        out=wv, in_=wvT.rearrange("(c p) f -> p c f", p=128))
                    bv = wpool.tile([128, 256], F32)
                    nc.scalar.dma_start(
                        out=bv, in_=bias_v[:, :].to_broadcast([128, 256]))
            wout = wpool.tile([128, 2, E], F16)
            nc.sync.dma_start(
                out=wout, in_=woutT.rearrange("(c p) f -> p c f", p=128))
            ones64 = wpool.tile([128, 64], F16)
            # named scope doubles as a compile-cache buster
            with nc.named_scope(f"init3_ldwopt{int(_LDWOPT)}_bc{_BCAST}"):
                nc.vector.memset(ones64, 1.0)
            onesbc = wpool.tile([1, 64], F16)
            nc.vector.tensor_copy(onesbc, ones64[0:1, :])

            # persistent activations
            qk = qkpool.tile([128, 4, S], F16)        # Q^T (chunks 0-1), K^T (2-3)
            # V natural + ones col, flattened per s-chunk to [4*65 + 68pad]
            # so each head's [V|1] stationary can be read as a 128-col AP
            # (full-width weights enable the PE fast weight load).
            va = vapool.tile([128, NSCH, 328], F16)
            attn = attnpool.tile([128, 2, S], F16)    # attn^T normalized

            def va_hd(i):
                return va[:, i, 0:260].rearrange("p (h c) -> p h c", h=4)

            nc.vector.memset(va[:, :, 260:328], 0.0)
            nc.vector.memset(
                va[:, :, 0:260].rearrange(
                    "p i (h c) -> p i h c", h=4)[:, :, :, 64:65], 1.0)

            with tc.tile_pool(name="apsum", bufs=2, space="PSUM") as apsum, \
                 tc.tile_pool(name="unpool", bufs=4) as unpool, \
                 tc.tile_pool(name="fpool", bufs=3) as fpool, \
                 tc.tile_pool(name="opool", bufs=6) as opool:

                def emit_qk(fc, sbs=range(NSB)):
                    with nc.named_scope(f"proj_qk{fc}"):
                        for sb in sbs:
                            ps = apsum.tile([128, 512], F32, tag="aps")
                            for ec in range(NEC):
                                nc.tensor.matmul(
                                    ps,
                                    wqk[:, ec, bass.ts(fc, 128)],
                                    xt[:, ec, bass.ts(sb, 512)],
                                    start=(ec == 0), stop=(ec == NEC - 1))
                            nc.vector.tensor_scalar(
                                out=qk[:, fc, bass.ts(sb, 512)], in0=ps,
                                scalar1=bqk[:, fc:fc + 1], scalar2=None,
                                op0=mybir.AluOpType.add)

                def emit_v(chunks=range(NSCH)):
                    with nc.named_scope("proj_v"):
                        for i in chunks:
                            ps = apsum.tile([128, 512], F32, tag="aps")
                            for ec in range(NEC):
                                nc.tensor.matmul(
                                    ps[:, 0:256],
                                    xt[:, ec, bass.ts(i, 128)],
                                    wv[:, ec, :],
                                    start=(ec == 0), stop=(ec == NEC - 1))
                            nc.vector.tensor_tensor(
                                out=va_hd(i)[:, :, 0:64],
                                in0=ps[:, 0:256].rearrange(
                                    "p (h d) -> p h d", h=4),
                                in1=bv.rearrange("p (h d) -> p h d", h=4),
                                op=mybir.AluOpType.add)

                def emit_oproj(pair, tq):
                    toff = tq * 512
                    final = (pair == 1 and tq == 3)
                    with nc.named_scope(f"oproj{pair}_{tq}"):
                        for fc in range(NEC):
                            ps = apsum.tile([128, 512], F32, tag="aps")
                            nc.tensor.matmul(
                                ps,
                                wout[:, pair, bass.ts(fc, 128)],
                                attn[:, pair, bass.ds(toff, 512)],
                                start=True, stop=True)
                            ocp = opool.tile([128, 512], F16, tag="ocp")
                            if final and fc % 2 == 1:
                                nc.scalar.copy(ocp, ps)
                            else:
                                nc.vector.tensor_copy(ocp, ps)
                            if final:
                                oeng = nc.sync if fc % 2 == 0 else nc.scalar
                            else:
                                oeng = nc.gpsimd
                            oeng.dma_start(
                                out=outT[pair, bass.ts(fc, 128),
                                         bass.ds(toff, 512)],
                                in_=ocp)

                def emit_pair(pair):
                    hA, hB = 2 * pair, 2 * pair + 1
                    qc = pair       # Q chunk of this pair
                    kc = 2 + pair   # K chunk
                    auxp = apsum
                    if True:
                        for tq in range(4):
                            toff = tq * 512
                            if tq > 0 and not (pair == 1 and tq == 3):
                                # out_proj for the previous t-quarter: its
                                # attn is ready by now, so the PE never
                                # stalls on the finalize chain. For the very
                                # last t-quarter it is instead emitted after
                                # the finalize, giving the PE work to chew
                                # while the closing reciprocal chain runs.
                                emit_oproj(pair, tq - 1)
                            pvA = pvp.tile([128, 512], F32, tag="pvA")
                            pvB = pvp.tile([128, 512], F32, tag="pvB")
                            # software-pipelined: scores/exp for i, P@V for
                            # i-1, so the PE streams scores while ACT runs.
                            ptiles = {}
                            with nc.named_scope(f"scores{pair}_{tq}"):
                                # two iterations per step: one scores->PV
                                # array handoff per TWO strips, and the four
                                # P@V matmuls chain back-to-back at stream
                                # rate. The exp ACT (2 strips in flight,
                                # scp/ppool double-buffered) stays the pacer.
                                for ib in range(0, NSCH + 2, 2):
                                    for i in (ib, ib + 1):
                                        if i >= NSCH:
                                            continue
                                        sc = scp.tile(
                                            [128, 1024], F32, tag="sc")
                                        nc.tensor.matmul(
                                            sc[:, 0:512],
                                            qk[0:64, kc, bass.ts(i, 128)],
                                            qk[0:64, qc, bass.ds(toff, 512)],
                                            start=True, stop=True)
                                        nc.tensor.matmul(
                                            sc[:, 512:1024],
                                            qk[64:128, kc, bass.ts(i, 128)],
                                            qk[64:128, qc, bass.ds(toff, 512)],
                                            start=True, stop=True)
                                        p = ppool.tile(
                                            [128, 1024], F16, tag="p")
                                        nc.scalar.activation(
                                            p, sc, EXP, scale=SCALING)
                                        ptiles[i] = p
                                    for i in (ib - 2, ib - 1):
                                        if i < 0 or i >= NSCH:
                                            continue
                                        pp = ptiles.pop(i)
                                        nc.tensor.matmul(
                                            pvA,
                                            va[:, i,
                                               hA * 65:hA * 65 + 128],
                                            pp[:, 0:512],
                                            start=(i == 0),
                                            stop=(i == NSCH - 1))
                                        nc.tensor.matmul(
                                            pvB,
                                            va[:, i,
                                               hB * 65:hB * 65 + 128],
                                            pp[:, 512:1024],
                                            start=(i == 0),
                                            stop=(i == NSCH - 1))
                            # finalize this t-quarter: stage unnormalized
                            # P@V + sums to SBUF (frees PSUM), reciprocal via
                            # partition shuffle, stride-0 DMA broadcast,
                            # normalize.
                            final = (pair == 1 and tq == 3)
                            with nc.named_scope(f"fin{pair}_{tq}"):
                                un = unpool.tile([65, 2, 512], F32, tag="un")
                                nc.vector.tensor_copy(un[:, 0, :], pvA[0:65, :])
                                nc.vector.tensor_copy(un[:, 1, :], pvB[0:65, :])
                                recin = fpool.tile(
                                    [128, 8], F32, tag="recin")
                                nc.sync.dma_start(
                                    out=recin, in_=un[64:65, :, :])
                                recw = fpool.tile([128, 8], F32, tag="recw")
                                nc.vector.reciprocal(recw, recin)
                                nc.sync.dma_start(
                                    out=recscr[tq % 2], in_=recw)
                                for h in range(2):
                                    prt = h * 64
                                    bcs = opool.tile(
                                        [64, 512], F32, tag="bcs")
                                    nc.sync.dma_start(
                                        out=bcs,
                                        in_=recscr[tq % 2, h:h + 1,
                                                   :].to_broadcast(
                                            [64, 512]))
                                    nc.vector.tensor_mul(
                                        attn[prt:prt + 64, pair,
                                             bass.ds(toff, 512)],
                                        un[0:64, h, :],
                                        bcs)

                emit_qk(2, range(2))
                emit_qk(0, range(2))
                emit_v(range(8))
                emit_qk(2, range(2, 4))
                emit_qk(0, range(2, 4))
                emit_v(range(8, NSCH))
                emit_pair(0)
                emit_qk(1)
                emit_qk(3)
                emit_oproj(0, 3)
                emit_pair(1)
                emit_oproj(1, 2)
                emit_oproj(1, 3)
    _split_excess_waits(nc)
    return nc


_NC_CACHE = None


def _get_nc():
    global _NC_CACHE
    if _NC_CACHE is None:
        _NC_CACHE = _build_nc()
    return _NC_CACHE


def kernel(x, in_proj_weight, in_proj_bias, out_proj_weight, out_proj_bias,
           _run_kwargs=None, _capture=None):
    x = np.asarray(x, dtype=np.float32)
    in_proj_weight = np.asarray(in_proj_weight, dtype=np.float32)
    in_proj_bias = np.asarray(in_proj_bias, dtype=np.float32)
    out_proj_weight = np.asarray(out_proj_weight, dtype=np.float32)
    out_proj_bias = np.asarray(out_proj_bias, dtype=np.float32)

    nc = _get_nc()
    xTb = [np.ascontiguousarray(x[:, b, :].T.astype(np.float16))
           for b in range(B)]

    in_maps = []
    for c in range(N_CORES):
        b = c // 4
        h0 = (c % 4) * 4
        rows = slice(h0 * D, h0 * D + 4 * D)
        wq = in_proj_weight[0:E][rows]          # [256, 1024]
        wk = in_proj_weight[E:2 * E][rows]
        wv_ = in_proj_weight[2 * E:3 * E][rows]
        wqkT = np.ascontiguousarray(
            np.concatenate([wq, wk], axis=0).T.astype(np.float16))
        wvT = np.ascontiguousarray(wv_.T.astype(np.float16))
        woutT = np.ascontiguousarray(
            out_proj_weight[:, rows].T.astype(np.float16))
        bqk = np.concatenate(
            [in_proj_bias[0:E][rows], in_proj_bias[E:2 * E][rows]])
        bias_qk = np.ascontiguousarray(bqk.reshape(4, 128).T)
        bias_v = in_proj_bias[2 * E:3 * E][rows].reshape(1, 256)
        in_maps.append({
            "xT": xTb[b],
            "wqkT": wqkT,
            "wvT": wvT,
            "woutT": woutT,
            "bias_qk": bias_qk,
            "bias_v": np.ascontiguousarray(bias_v),
        })

    res = run_bass_kernel_spmd(nc, in_maps, core_ids=list(range(N_CORES)),
                               **(_run_kwargs or {}))
    if _capture is not None:
        _capture["res"] = res

    out = np.zeros((S, B, E), dtype=np.float32)
    for c in range(N_CORES):
        b = c // 4
        o = res.results[c]["outT"]
        out[:, b, :] += o[0].T.astype(np.float32)
        out[:, b, :] += o[1].T.astype(np.float32)
    out += out_proj_bias
    return out


# revision 26
# speedup vs baseline: 1.0365x; 1.0313x over previous
"""MultiHeadAttention forward on 8 TRN2 NeuronCores (batch*head parallel).

Problem: S=2048, B=2, E=1024, H=16 heads, D=64. Each core handles one batch
(b = core//4) and 4 consecutive heads ((core%4)*4 ...), as 2 head-pairs.

Final version (286us -> ~230us): fp16 matmul operands everywhere (1 cyc/row
on the PE like bf16 but with a 10-bit mantissa; measured f32r streams ~1.6
cyc/row on HW), fp16 partial outputs (halves the out-DMA), deferred-PV
software pipelining (the PE streams scores(i+1) while the ACT exps strip i,
never stalling on the exp), out_proj deferred one t-quarter (the PE never
waits on the finalize chain), input DMAs split across the SP/Activation HWDGE
queues with wqk first and x in column halves, steady-state out-DMAs issued
from the idle GpSimd SWDGE queue, and the reciprocal broadcast done with a
DRAM-bounce stride-0 DMA. PSUM accumulation stays fp32 throughout.

Measured phase-B steady state: score pair 31ns apart (row groups stream
concurrently), P@V pair chains at ~215ns/512 rows; ~1.04us/iteration vs the
1.11us exp ACT — the kernel is tensor-engine-bound end-to-end at ~80% PE
occupancy, with the remainder being input-DMA ramp (~14us), the closing
reciprocal/out_proj chain (~10us), and the final sync barrier (~7us).

Per-core program:
  Phase A: QKV projection. Host pre-transposes x and weight slices so the
    contraction dim (E) lands on SBUF partitions. Q^T/K^T in feature-major
    [f, s] layout; V in natural [s, d] layout with an appended ones column
    (the softmax denominator drops out of the P@V matmul). Only pair 0's
    Q/K + V precede phase B; pair 1's Q/K are emitted between the pairs.
  Phase B: per head-pair, per t-quarter (512): row-packed K=64 score matmuls
    (heads at array rows 0-63/64-127 run concurrently), one ACT exp over the
    combined [128, 1024] PSUM strip (scale=1/8 folded in) producing fp16 P,
    then P@V accumulation with [V|1] stationary deferred one iteration so
    the PE streams scores(i+1) while the ACT runs exp(i).
  Finalize (per pair/tq, off critical path): PSUM->SBUF staging copy,
    batched DVE reciprocal over the 8 denominator rows (partition-shuffled
    via DMA), broadcast via stride-0 DMA, DVE multiply (casts attn to fp16).
  Phase C: out_proj partials per pair; host sums the 2x4 fp16 partials per
    batch in fp32 and adds out_proj_bias.
"""
import os
import sys

if "/opt/trn_rl_repo" not in sys.path:
    sys.path.insert(0, "/opt/trn_rl_repo")

import numpy as np

import concourse.bass as bass
import concourse.tile as tile
from concourse import mybir
from concourse.bass_utils import run_bass_kernel_spmd

_LDWOPT = os.environ.get("LDWOPT", "0") == "1"
if _LDWOPT:
    import concourse.bass_utils as _bu

    _orig_run_command = _bu.run_command

    def _run_command_ldwopt(argv, **kw):
        argv = ["--enable-ldw-opt=true" if a == "--enable-ldw-opt=false" else a
                for a in argv]
        return _orig_run_command(argv, **kw)

    _bu.run_command = _run_command_ldwopt

# BCAST mode: "dma" = stride-0 SBUF->SBUF DMA; "pe" = ones-matmul on the PE.
_BCAST = os.environ.get("BCAST", "dma")

S = 2048
B = 2
E = 1024
H = 16
D = 64
N_CORES = 8
F32 = mybir.dt.float32
F16 = mybir.dt.float16
EXP = mybir.ActivationFunctionType.Exp
SCALING = float(D) ** -0.5

NSCH = S // 128   # 16 s-chunks
NSB = S // 512    # 4 s-blocks
NEC = E // 128    # 8 e-chunks


def _split_excess_waits(nc, limit=1):
    """This walrus build accepts at most 2 sync-wait commands per instruction;
    hoist excess waits onto preceding same-engine NOPs (queue order preserves
    semantics)."""
    ctr = 0
    for f in nc.m.functions:
        for blk in f.blocks:
            insts = blk.instructions
            if not any(
                i.sync_info and i.sync_info.on_wait and len(i.sync_info.on_wait) > limit
                for i in insts
            ):
                continue
            out = []
            for inst in insts:
                si = inst.sync_info
                if si and si.on_wait and len(si.on_wait) > limit:
                    waits = list(si.on_wait)
                    excess, keep = waits[:-limit], waits[-limit:]
                    for i in range(0, len(excess), limit):
                        ctr += 1
                        nop = mybir.InstNoOp(name=f"waitsplit-nop-{ctr}")
                        nop.engine = inst.engine
                        nop.sync_info = mybir.SyncInfo(
                            on_wait=excess[i : i + limit], on_update=[]
                        )
                        nc.register_instruction(nop, overwrite=True)
                        out.append(nop)
                    si.on_wait = keep
                out.append(inst)
            blk.instructions.clear()
            blk.instructions.extend(out)
    return nc


def _build_nc():
    nc = bass.Bass()
    xT = nc.dram_tensor("xT", [E, S], F16, kind="ExternalInput")
    wqkT = nc.dram_tensor("wqkT", [E, 512], F16, kind="ExternalInput")
    wvT = nc.dram_tensor("wvT", [E, 256], F16, kind="ExternalInput")
    woutT = nc.dram_tensor("woutT", [256, E], F16, kind="ExternalInput")
    bias_qk = nc.dram_tensor("bias_qk", [128, 4], F32, kind="ExternalInput")
    bias_v = nc.dram_tensor("bias_v", [1, 256], F32, kind="ExternalInput")
    outT = nc.dram_tensor("outT", [2, E, S], F16, kind="ExternalOutput")
    recscr = nc.dram_tensor("recscr", [2, 2, 512], F32, kind="Internal")

    with tile.TileContext(nc) as tc:
        with tc.tile_pool(name="wpool", bufs=1) as wpool, \
             tc.tile_pool(name="qkpool", bufs=1) as qkpool, \
             tc.tile_pool(name="vapool", bufs=1) as vapool, \
             tc.tile_pool(name="attnpool", bufs=1) as attnpool, \
             tc.tile_pool(name="ppool", bufs=4) as ppool, \
             tc.tile_pool(name="scpsum", bufs=2, space="PSUM") as scp, \
             tc.tile_pool(name="pvpsum", bufs=1, space="PSUM") as pvp:
            # ---- constants / weights. x + QK weights first (the PE's
            # critical path); spread across issue queues so transfers overlap.
            xt = wpool.tile([128, NEC, S], F16)
            wqk = wpool.tile([128, NEC, 512], F16)
            # wqk first (small, needed by every projection group), then x in
            # column-halves so s-blocks 0-1 can project before x fully lands.
            # wv/bv must land before the V chunks (3rd in PE order).
            for ec in range(NEC):
                eng = nc.sync if ec % 2 == 0 else nc.scalar
                eng.dma_start(
                    out=wqk[:, ec, :], in_=wqkT[bass.ts(ec, 128), :])
            bqk = wpool.tile([128, 4], F32)
            nc.scalar.dma_start(out=bqk, in_=bias_qk[:, :])
            for half in range(2):
                cs = bass.ds(half * 1024, 1024)
                for ec in range(NEC):
                    eng = nc.sync if ec % 2 == 0 else nc.scalar
                    eng.dma_start(out=xt[:, ec, cs],
                                  in_=xT[bass.ts(ec, 128), cs])
                if half == 0:
                    wv = wpool.tile([128, NEC, 256], F16)
                    nc.scalar.dma_start(
                        out=wv, in_=wvT.rearrange("(c p) f -> p c f", p=128))
                    bv = wpool.tile([128, 256], F32)
                    nc.scalar.dma_start(
                        out=bv, in_=bias_v[:, :].to_broadcast([128, 256]))
            wout = wpool.tile([128, 2, E], F16)
            nc.sync.dma_start(
                out=wout, in_=woutT.rearrange("(c p) f -> p c f", p=128))
            ones64 = wpool.tile([128, 64], F16)
            # named scope doubles as a compile-cache buster
            with nc.named_scope(f"init3_ldwopt{int(_LDWOPT)}_bc{_BCAST}"):
                nc.vector.memset(ones64, 1.0)
            onesbc = wpool.tile([1, 64], F16)
            nc.vector.tensor_copy(onesbc, ones64[0:1, :])

            # persistent activations
            qk = qkpool.tile([128, 4, S], F16)        # Q^T (chunks 0-1), K^T (2-3)
            # V natural + ones col, flattened per s-chunk to [4*65 + 68pad]
            # so each head's [V|1] stationary can be read as a 128-col AP
            # (full-width weights enable the PE fast weight load).
            va = vapool.tile([128, NSCH, 328], F16)
            attn = attnpool.tile([128, 2, S], F16)    # attn^T normalized

            def va_hd(i):
                return va[:, i, 0:260].rearrange("p (h c) -> p h c", h=4)

            nc.vector.memset(va[:, :, 260:328], 0.0)
            nc.vector.memset(
                va[:, :, 0:260].rearrange(
                    "p i (h c) -> p i h c", h=4)[:, :, :, 64:65], 1.0)

            with tc.tile_pool(name="apsum", bufs=2, space="PSUM") as apsum, \
                 tc.tile_pool(name="unpool", bufs=4) as unpool, \
                 tc.tile_pool(name="fpool", bufs=3) as fpool, \
                 tc.tile_pool(name="opool", bufs=6) as opool:

                def emit_qk(fc, sbs=range(NSB)):
                    with nc.named_scope(f"proj_qk{fc}"):
                        for sb in sbs:
                            ps = apsum.tile([128, 512], F32, tag="aps")
                            for ec in range(NEC):
                                nc.tensor.matmul(
                                    ps,
                                    wqk[:, ec, bass.ts(fc, 128)],
                                    xt[:, ec, bass.ts(sb, 512)],
                                    start=(ec == 0), stop=(ec == NEC - 1))
                            nc.vector.tensor_scalar(
                                out=qk[:, fc, bass.ts(sb, 512)], in0=ps,
                                scalar1=bqk[:, fc:fc + 1], scalar2=None,
                                op0=mybir.AluOpType.add)

                def emit_v(chunks=range(NSCH)):
                    with nc.named_scope("proj_v"):
                        for i in chunks:
                            ps = apsum.tile([128, 512], F32, tag="aps")
                            for ec in range(NEC):
                                nc.tensor.matmul(
                                    ps[:, 0:256],
                                    xt[:, ec, bass.ts(i, 128)],
                                    wv[:, ec, :],
                                    start=(ec == 0), stop=(ec == NEC - 1))
                            nc.vector.tensor_tensor(
                                out=va_hd(i)[:, :, 0:64],
                                in0=ps[:, 0:256].rearrange(
                                    "p (h d) -> p h d", h=4),
                                in1=bv.rearrange("p (h d) -> p h d", h=4),
                                op=mybir.AluOpType.add)

                def emit_oproj(pair, tq):
                    toff = tq * 512
                    final = (pair == 1 and tq == 3)
                    with nc.named_scope(f"oproj{pair}_{tq}"):
                        for fc in range(NEC):
                            ps = apsum.tile([128, 512], F32, tag="aps")
                            nc.tensor.matmul(
                                ps,
                                wout[:, pair, bass.ts(fc, 128)],
                                attn[:, pair, bass.ds(toff, 512)],
                                start=True, stop=True)
                            ocp = opool.tile([128, 512], F16, tag="ocp")
                            if final and fc % 2 == 1:
                                nc.scalar.copy(ocp, ps)
                            else:
                                nc.vector.tensor_copy(ocp, ps)
                            if final:
                                oeng = nc.sync if fc % 2 == 0 else nc.scalar
                            else:
                                oeng = nc.gpsimd
                            oeng.dma_start(
                                out=outT[pair, bass.ts(fc, 128),
                                         bass.ds(toff, 512)],
                                in_=ocp)

                def emit_pair(pair, fillers=()):
                    hA, hB = 2 * pair, 2 * pair + 1
                    qc = pair       # Q chunk of this pair
                    kc = 2 + pair   # K chunk
                    auxp = apsum
                    if True:
                        for tq in range(4):
                            toff = tq * 512
                            if tq > 0 and not (pair == 1 and tq == 3):
                                # out_proj for the previous t-quarter: its
                                # attn is ready by now, so the PE never
                                # stalls on the finalize chain. For the very
                                # last t-quarter it is instead emitted after
                                # the finalize, giving the PE work to chew
                                # while the closing reciprocal chain runs.
                                emit_oproj(pair, tq - 1)
                            pvA = pvp.tile([128, 512], F32, tag="pvA")
                            pvB = pvp.tile([128, 512], F32, tag="pvB")
                            # software-pipelined: scores/exp for i, P@V for
                            # i-1, so the PE streams scores while ACT runs.
                            ptiles = {}
                            with nc.named_scope(f"scores{pair}_{tq}"):
                                # two iterations per step: one scores->PV
                                # array handoff per TWO strips, and the four
                                # P@V matmuls chain back-to-back at stream
                                # rate. The exp ACT (2 strips in flight,
                                # scp/ppool double-buffered) stays the pacer.
                                for ib in range(0, NSCH + 2, 2):
                                    for i in (ib, ib + 1):
                                        if i >= NSCH:
                                            continue
                                        sc = scp.tile(
                                            [128, 1024], F32, tag="sc")
                                        nc.tensor.matmul(
                                            sc[:, 0:512],
                                            qk[0:64, kc, bass.ts(i, 128)],
                                            qk[0:64, qc, bass.ds(toff, 512)],
                                            start=True, stop=True)
                                        nc.tensor.matmul(
                                            sc[:, 512:1024],
                                            qk[64:128, kc, bass.ts(i, 128)],
                                            qk[64:128, qc, bass.ds(toff, 512)],
                                            start=True, stop=True)
                                        p = ppool.tile(
                                            [128, 1024], F16, tag="p")
                                        nc.scalar.activation(
                                            p, sc, EXP, scale=SCALING)
                                        ptiles[i] = p
                                    for i in (ib - 2, ib - 1):
                                        if i < 0 or i >= NSCH:
                                            continue
                                        pp = ptiles.pop(i)
                                        nc.tensor.matmul(
                                            pvA,
                                            va[:, i,
                                               hA * 65:hA * 65 + 128],
                                            pp[:, 0:512],
                                            start=(i == 0),
                                            stop=(i == NSCH - 1))
                                        nc.tensor.matmul(
                                            pvB,
                                            va[:, i,
                                               hB * 65:hB * 65 + 128],
                                            pp[:, 512:1024],
                                            start=(i == 0),
                                            stop=(i == NSCH - 1))
                            # finalize this t-quarter: stage unnormalized
                            # P@V + sums to SBUF (frees PSUM), reciprocal via
                            # partition shuffle, stride-0 DMA broadcast,
                            # normalize.
                            if tq < len(fillers):
                                # absorb one projection group of the next
                                # pair into this window's PE slack (the ACT
                                # keeps running off its 2-strip lookahead).
                                fillers[tq]()
                            final = (pair == 1 and tq == 3)
                            with nc.named_scope(f"fin{pair}_{tq}"):
                                un = unpool.tile([65, 2, 512], F32, tag="un")
                                nc.vector.tensor_copy(un[:, 0, :], pvA[0:65, :])
                                nc.vector.tensor_copy(un[:, 1, :], pvB[0:65, :])
                                recin = fpool.tile(
                                    [128, 8], F32, tag="recin")
                                nc.sync.dma_start(
                                    out=recin, in_=un[64:65, :, :])
                                recw = fpool.tile([128, 8], F32, tag="recw")
                                nc.vector.reciprocal(recw, recin)
                                nc.sync.dma_start(
                                    out=recscr[tq % 2], in_=recw)
                                for h in range(2):
                                    prt = h * 64
                                    bcs = opool.tile(
                                        [64, 512], F32, tag="bcs")
                                    nc.sync.dma_start(
                                        out=bcs,
                                        in_=recscr[tq % 2, h:h + 1,
                                                   :].to_broadcast(
                                            [64, 512]))
                                    nc.vector.tensor_mul(
                                        attn[prt:prt + 64, pair,
                                             bass.ds(toff, 512)],
                                        un[0:64, h, :],
                                        bcs)

                emit_qk(2, range(2))
                emit_qk(0, range(2))
                emit_v(range(8))
                emit_qk(2, range(2, 4))
                emit_qk(0, range(2, 4))
                emit_v(range(8, NSCH))
                emit_pair(0, fillers=[
                    lambda: emit_qk(1, range(0, 1)),
                    lambda: emit_qk(1, range(1, 2)),
                    lambda: emit_qk(3, range(0, 1)),
                    lambda: emit_qk(3, range(1, 2)),
                ])
                emit_qk(1, range(2, 4))
                emit_qk(3, range(2, 4))
                emit_oproj(0, 3)
                emit_pair(1)
                emit_oproj(1, 2)
                emit_oproj(1, 3)
    _split_excess_waits(nc)
    return nc


_NC_CACHE = None


def _get_nc():
    global _NC_CACHE
    if _NC_CACHE is None:
        _NC_CACHE = _build_nc()
    return _NC_CACHE


def kernel(x, in_proj_weight, in_proj_bias, out_proj_weight, out_proj_bias,
           _run_kwargs=None, _capture=None):
    x = np.asarray(x, dtype=np.float32)
    in_proj_weight = np.asarray(in_proj_weight, dtype=np.float32)
    in_proj_bias = np.asarray(in_proj_bias, dtype=np.float32)
    out_proj_weight = np.asarray(out_proj_weight, dtype=np.float32)
    out_proj_bias = np.asarray(out_proj_bias, dtype=np.float32)

    nc = _get_nc()
    xTb = [np.ascontiguousarray(x[:, b, :].T.astype(np.float16))
           for b in range(B)]

    in_maps = []
    for c in range(N_CORES):
        b = c // 4
        h0 = (c % 4) * 4
        rows = slice(h0 * D, h0 * D + 4 * D)
        wq = in_proj_weight[0:E][rows]          # [256, 1024]
        wk = in_proj_weight[E:2 * E][rows]
        wv_ = in_proj_weight[2 * E:3 * E][rows]
        wqkT = np.ascontiguousarray(
            np.concatenate([wq, wk], axis=0).T.astype(np.float16))
        wvT = np.ascontiguousarray(wv_.T.astype(np.float16))
        woutT = np.ascontiguousarray(
            out_proj_weight[:, rows].T.astype(np.float16))
        bqk = np.concatenate(
            [in_proj_bias[0:E][rows], in_proj_bias[E:2 * E][rows]])
        bias_qk = np.ascontiguousarray(bqk.reshape(4, 128).T)
        bias_v = in_proj_bias[2 * E:3 * E][rows].reshape(1, 256)
        in_maps.append({
            "xT": xTb[b],
            "wqkT": wqkT,
            "wvT": wvT,
            "woutT": woutT,
            "bias_qk": bias_qk,
            "bias_v": np.ascontiguousarray(bias_v),
        })

    res = run_bass_kernel_spmd(nc, in_maps, core_ids=list(range(N_CORES)),
                               **(_run_kwargs or {}))
    if _capture is not None:
        _capture["res"] = res

    out = np.zeros((S, B, E), dtype=np.float32)
    for c in range(N_CORES):
        b = c // 4
        o = res.results[c]["outT"]
        out[:, b, :] += o[0].T.astype(np.float32)
        out[:, b, :] += o[1].T.astype(np.float32)
    out += out_proj_bias
    return out
